# revision 3
# baseline (speedup 1.0000x reference)
"""Trainium2 Bass kernel for DSAttention (causal attention with per-batch tau
scale and per-key delta bias), B=4 L=S=2048 H=8 E=64 fp32.

Strategy: data+head parallelism across 8 cores (core i -> batch i//2, heads
(i%2)*4 .. +4, i.e. 4 (b,h) pairs per core). Per pair, flash-style attention
computed entirely in the transposed-score layout:

  scoresT[s, l] = sum_e K[s,e] Q[l,e]          (PE, float32r, K/Q pre-transposed
                                                on host so E sits on partitions)
  AT[s, l] = exp((tau/8)*scoresT + delta[s]/8) (ACT; tau via per-partition scale
                                                AP, delta via per-partition bias
                                                AP; no max-subtraction needed --
                                                scores are bounded ~|9|)
  OT[e', l] += V'[s, e'].T @ AT[s, l]          (PE accumulates over s-tiles in
                                                PSUM; V' has a ones column so
                                                row 64 of OT is the softmax
                                                denominator)
  out[l, e] = OT[e, l] / OT[64, l]             (PE transpose of OT chunks, then
                                                DVE reciprocal + scalar mul)

Causality: s-tile si only contributes to columns l >= 128*si; the triangular
diagonal block is masked by adding -1e30 before the exp.
"""

import sys

if "/opt/trn_rl_repo" not in sys.path:
    sys.path.insert(0, "/opt/trn_rl_repo")

import numpy as np

import concourse.bacc as bacc
import concourse.mybir as mybir
import concourse.tile as tile
from concourse import bass_utils

B, L, S, H, E = 4, 2048, 2048, 8, 64
N_CORES = 8
PAIRS = 4          # (b, h) pairs per core
NT = S // 128      # s-tiles per pair
CHUNK = 1024       # scoresT psum chunk width (2 PSUM banks)
F32 = mybir.dt.float32
F32R = mybir.dt.float32r
NEG = -1.0e30


def _pieces(lo, hi):
    """Split [lo, hi) at absolute 512 boundaries (PSUM bank alignment)."""
    out = []
    a = lo
    while a < hi:
        b = min(hi, (a // 512 + 1) * 512)
        out.append((a, b))
        a = b
    return out


def _body(tc, nc, qt_d, kt_d, vv_d, deltas_d, taub_d, mask_d, eye_d, out_d):
    Exp = mybir.ActivationFunctionType.Exp
    with (
        tc.tile_pool(name="const", bufs=1) as cp,
        tc.tile_pool(name="io", bufs=2) as iop,
        tc.tile_pool(name="at", bufs=3) as atp,
        tc.tile_pool(name="outp", bufs=2) as op,
        tc.tile_pool(name="ps_s", bufs=2, space="PSUM") as pss,
        tc.tile_pool(name="ps_ot", bufs=1, space="PSUM") as psot,
    ):
        # ---- constants (per core; the core's 4 pairs share one batch b) ----
        delta_raw = cp.tile([128, NT], F32)
        nc.sync.dma_start(delta_raw[:], deltas_d.ap())
        delta_sc = cp.tile([128, NT], F32)
        nc.vector.tensor_scalar_mul(delta_sc[:], delta_raw[:], 0.125)
        tau_raw = cp.tile([128, 1], F32)
        nc.sync.dma_start(tau_raw[:], taub_d.ap())
        tau_sc = cp.tile([128, 1], F32)
        nc.vector.tensor_scalar_mul(tau_sc[:], tau_raw[:], 0.125)
        mask_t = cp.tile([128, 128], F32)
        nc.sync.dma_start(mask_t[:], mask_d.ap())
        eye_t = cp.tile([65, 65], F32)
        nc.sync.dma_start(eye_t[:], eye_d.ap())

        for p in range(PAIRS):
            qt_t = iop.tile([E, L], F32R, tag="qt")
            nc.sync.dma_start(qt_t[:], qt_d.ap()[p])
            # fold tau/8 into Q (avoids the activation scale-as-AP path)
            nc.vector.tensor_scalar_mul(qt_t[:], qt_t[:], tau_sc[0:E, 0:1])
            kt_t = iop.tile([E, S], F32R, tag="kt")
            nc.sync.dma_start(kt_t[:], kt_d.ap()[p])
            # V' tiles: [128, si, 65]; col 64 (ones) comes appended from host
            vp_t = iop.tile([128, NT * 65], F32R, tag="vp")
            vp3 = vp_t[:].rearrange("p (s c) -> p s c", c=65)
            nc.sync.dma_start(
                vp3[:, :, :], vv_d.ap()[p].rearrange("(s p) e -> p s e", p=128)
            )

            ot = psot.tile([65, L], F32)  # 4 PSUM banks, accumulates over si
            for si in range(NT):
                l0 = si * 128
                for c in range(l0 // CHUNK, L // CHUNK):
                    cl = c * CHUNK
                    cr = cl + CHUNK
                    lo = max(l0, cl)
                    st = pss.tile([128, CHUNK], F32, tag="st")
                    for a, b in _pieces(lo, cr):
                        nc.tensor.matmul(
                            st[:, a - cl : b - cl],
                            kt_t[:, l0 : l0 + 128],
                            qt_t[:, a:b],
                            start=True,
                            stop=True,
                        )
                    if cl <= l0 < cr:  # mask the triangular diagonal block
                        od = l0 - cl
                        nc.vector.tensor_add(
                            st[:, od : od + 128], st[:, od : od + 128], mask_t[:]
                        )
                    at = atp.tile([128, CHUNK], F32R, tag="at")
                    w = cr - lo
                    nc.scalar.activation(
                        at[:, 0:w],
                        st[:, lo - cl : CHUNK],
                        Exp,
                        bias=delta_sc[:, si : si + 1],
                        scale=1.0,
                    )
                    for a, b in _pieces(lo, cr):
                        bank = a // 512
                        nc.tensor.matmul(
                            ot[:, a:b],
                            vp3[:, si, :],
                            at[:, a - lo : b - lo],
                            start=(si == 0),
                            stop=(si == 4 * bank + 3),
                        )

            # ---- tail: normalize + transpose back + store ----
            ot_sb = op.tile([65, L], F32, tag="otsb")
            nc.vector.tensor_copy(ot_sb[:], ot[:])
            for t in range(L // 128):
                tr = pss.tile([128, 65], F32, tag="st")
                nc.tensor.transpose(tr[:], ot_sb[:, 128 * t : 128 * (t + 1)], eye_t[:])
                r = op.tile([128, 1], F32, tag="r")
                nc.vector.reciprocal(r[:], tr[:, 64:65])
                o_sb = op.tile([128, 64], F32, tag="o")
                nc.vector.tensor_scalar_mul(o_sb[:], tr[:, 0:64], r[:, 0:1])
                nc.sync.dma_start(out_d.ap()[p, 128 * t : 128 * (t + 1), :], o_sb[:])


_CACHE = {}


def _build():
    if "nc" in _CACHE:
        return _CACHE["nc"]
    nc = bacc.Bacc("TRN2", target_bir_lowering=False, debug=False, num_devices=N_CORES)
    qt_d = nc.dram_tensor("qt", [PAIRS, E, L], F32R, kind="ExternalInput")
    kt_d = nc.dram_tensor("kt", [PAIRS, E, S], F32R, kind="ExternalInput")
    vv_d = nc.dram_tensor("vv", [PAIRS, S, E + 1], F32R, kind="ExternalInput")
    deltas_d = nc.dram_tensor("deltas", [128, NT], F32, kind="ExternalInput")
    taub_d = nc.dram_tensor("taub", [128, 1], F32, kind="ExternalInput")
    mask_d = nc.dram_tensor("mask", [128, 128], F32, kind="ExternalInput")
    eye_d = nc.dram_tensor("eye", [65, 65], F32, kind="ExternalInput")
    out_d = nc.dram_tensor("out", [PAIRS, L, E], F32, kind="ExternalOutput")
    with tile.TileContext(nc) as tc:
        _body(tc, nc, qt_d, kt_d, vv_d, deltas_d, taub_d, mask_d, eye_d, out_d)
    nc.compile()
    _CACHE["nc"] = nc
    return nc


def _in_maps(queries, keys, values, tau, delta):
    qt = np.ascontiguousarray(queries.transpose(0, 2, 3, 1))  # [B, H, E, L]
    kt = np.ascontiguousarray(keys.transpose(0, 2, 3, 1))
    vv = np.concatenate(
        [values.transpose(0, 2, 1, 3), np.ones((B, H, S, 1), np.float32)], axis=3
    )  # [B, H, S, E+1] with ones column for the softmax denominator
    # mask[s, l] = 0 if l >= s else -1e30 (transposed-layout causal mask)
    mask = np.where(
        np.arange(128)[None, :] >= np.arange(128)[:, None], 0.0, NEG
    ).astype(np.float32)
    eye = np.eye(65, dtype=np.float32)
    maps = []
    for i in range(N_CORES):
        b, h0 = i // 2, (i % 2) * PAIRS
        maps.append(
            {
                "qt": qt[b, h0 : h0 + PAIRS],
                "kt": kt[b, h0 : h0 + PAIRS],
                "vv": vv[b, h0 : h0 + PAIRS],
                "deltas": np.ascontiguousarray(
                    delta[b].reshape(NT, 128).T.astype(np.float32)
                ),
                "taub": np.full((128, 1), tau[b, 0], dtype=np.float32),
                "mask": mask,
                "eye": eye,
            }
        )
    return maps


def kernel(queries, keys, values, tau, delta, trace=False, trace_cores=None):
    queries = np.asarray(queries, dtype=np.float32)
    keys = np.asarray(keys, dtype=np.float32)
    values = np.asarray(values, dtype=np.float32)
    tau = np.asarray(tau, dtype=np.float32)
    delta = np.asarray(delta, dtype=np.float32)

    nc = _build()
    maps = _in_maps(queries, keys, values, tau, delta)
    res = bass_utils.run_bass_kernel_spmd(
        nc,
        maps,
        core_ids=list(range(N_CORES)),
        trace=trace,
        trace_cores=trace_cores,
    )
    out = np.empty((B, L, H, E), dtype=np.float32)
    for i in range(N_CORES):
        b, h0 = i // 2, (i % 2) * PAIRS
        o = res.results[i]["out"]  # [PAIRS, L, E]
        for j in range(PAIRS):
            out[b, :, h0 + j, :] = o[j]
    if trace:
        return out, res
    return out
